# revision 24
# baseline (speedup 1.0000x reference)
"""Causal single-head attention (B=8, S=2048, D=1024, fp32) on 8 NeuronCores.

Data-parallel over batch: one batch element per core, weights replicated.

Key algebraic trick: S = Q K^T = x (Wq Wk^T) x^T = A x^T with A = x M and
M = Wq Wk^T (1024x1024, computed once per core, ~2.1 GFLOP vs 4.3 GFLOP for
the K projection it replaces). K/Q are never materialized, and everything
(x^T, A^T, V) stays SBUF-resident in bf16 — no DRAM scratch traffic at all.

Per-core pipeline (matmuls bf16 @ 1 cycle/row, transposes fp32r):
  1. xT = x^T, WqT = Wq^T, WkT = Wk^T via PE transposes (cast bf16 on copy-out)
  2. M[d,e] = sum_t Wq[d,t] Wk[e,t]    (lhsT=WqT slices, rhs=WkT)
  3. AT = (x M)^T  (lhsT=M slices, rhs=xT) ; V = x Wv (lhsT=xT, rhs=Wv bf16)
  4. per 512-wide query chunk c, k-tile block:
       S^T[k, q-chunk] over 8 e-tiles of xT/AT (diagonal blocks width-trimmed)
       P^T = exp(S^T / 32) on ScalarE -> bf16; diagonal 128x128 triangle
       masked by a 0/1 multiply on DVE
       per q-tile j: rowsum / O accumulated interleaved (same est lhsT for
       O-halves and rowsum), O = (P^T.T @ V) * (1/rowsum)
"""

import numpy as np

B, S, D = 8, 2048, 1024
P = 128
NCORES = 8

_built = None


def _build():
    import ml_dtypes
    import concourse.tile as tile
    import concourse.mybir as mybir
    from concourse import bacc

    FP32 = mybir.dt.float32
    FP32R = mybir.dt.float32r
    BF16 = mybir.dt.bfloat16
    AF = mybir.ActivationFunctionType

    nc = bacc.Bacc("TRN2", target_bir_lowering=False, debug=False, num_devices=NCORES)
    x_d = nc.dram_tensor("x", [S, D], FP32R, kind="ExternalInput").ap()
    wq_d = nc.dram_tensor("Wq", [D, D], FP32R, kind="ExternalInput").ap()
    wk_d = nc.dram_tensor("Wk", [D, D], FP32R, kind="ExternalInput").ap()
    wv_d = nc.dram_tensor("Wv", [D, D], FP32, kind="ExternalInput").ap()
    out_d = nc.dram_tensor("out", [S, D], FP32, kind="ExternalOutput").ap()

    ident_c = nc.inline_tensor(np.eye(P, dtype=np.float32), name="ident_c")
    # [tri01 (128) | ones (2) | warm data (512)] in bf16
    tri = (np.arange(P)[None, :] >= np.arange(P)[:, None]).astype(np.float32)
    cst_np = np.concatenate([tri, np.ones((P, 2 + 512), np.float32)], axis=1)
    cst_c = nc.inline_tensor(cst_np.astype(ml_dtypes.bfloat16), name="cst_c")

    with tile.TileContext(nc) as tc:
        with (
            tc.tile_pool(name="bigs", bufs=1) as bigs,
            tc.tile_pool(name="sh5", bufs=40) as sh5,    # wqT/wkT, M, then est ring
            tc.tile_pool(name="s4", bufs=5) as s4,       # x staging, then osb
            tc.tile_pool(name="wst", bufs=4) as wst,     # W fp32 staging (pairs)
            tc.tile_pool(name="smalls", bufs=1) as smalls,
            tc.tile_pool(name="rcp", bufs=2) as rcp,
            tc.tile_pool(name="ps", bufs=6, space="PSUM") as ps,
            tc.tile_pool(name="rsp", bufs=2, space="PSUM") as rsp,
        ):
            ident = smalls.tile([P, P], FP32R, tag="ident")
            nc.sync.dma_start(out=ident, in_=ident_c.ap().bitcast(FP32R))
            cst = smalls.tile([P, 642], BF16, tag="cst")
            nc.scalar.dma_start(out=cst, in_=cst_c.ap())
            tri16 = cst[:, 0:128]
            ones16 = cst[:, 128:130]
            warm16 = cst[:, 130:642]

            xt = bigs.tile([P, 8, S], BF16, tag="xt")
            at = bigs.tile([P, 8, S], BF16, tag="at")
            vsb = bigs.tile([P, 16, D], BF16, tag="vsb")
            wv16 = bigs.tile([P, 8, D], BF16, tag="wv16")

            # ---- input DMA issue (3 queues) ----
            x_pend = {}
            w_pend = {}

            def load_x(si, eng):
                t = s4.tile([P, D], FP32R, tag="s4")
                eng.dma_start(out=t, in_=x_d[si * P:(si + 1) * P, :])
                x_pend[si] = t

            def load_w(which, kd2, eng):
                src, dt_ = {
                    "q": (wq_d, FP32R), "k": (wk_d, FP32R), "v": (wv_d, FP32),
                }[which]
                t = wst.tile([P, 2, D], dt_, tag="wst")
                # Wq/Wk: each partition gets two contiguous DRAM rows (8KB
                # descriptors, 2x queue rate); the d-interleave is undone in
                # the transpose copy-outs. Wv keeps the plain layout.
                pat = "(ki two) t -> ki two t" if which != "v" else \
                    "(two ki) t -> ki two t"
                eng.dma_start(
                    out=t,
                    in_=src[kd2 * 2 * P:(kd2 + 1) * 2 * P, :].rearrange(
                        pat, two=2
                    ) if which != "v" else
                    src[kd2 * 2 * P:(kd2 + 1) * 2 * P, :].rearrange(
                        pat, ki=P
                    ),
                )
                w_pend[(which, kd2)] = t

            # copy engines alternate
            cp_i = [0]

            def cp_eng():
                cp_i[0] += 1
                return (nc.vector, nc.scalar)[cp_i[0] % 2]

            def copy_cast(out, in_):
                e = cp_eng()
                if e is nc.vector:
                    e.tensor_copy(out=out, in_=in_)
                else:
                    e.copy(out=out, in_=in_)

            # ---- PE phases ----
            def warmup(rounds):
                for _ in range(rounds):
                    dps = ps.tile([P, 512], FP32, tag="ps")
                    for r in range(8):
                        nc.tensor.matmul(
                            dps, lhsT=tri16, rhs=warm16,
                            start=(r == 0), stop=(r == 7),
                        )

            def x_transpose_group(g):
                xts = [x_pend.pop(si) for si in range(4 * g, 4 * g + 4)]
                for kd in range(8):
                    tp4 = ps.tile([P, 512], FP32R, tag="ps")
                    for j in range(4):
                        nc.tensor.matmul(
                            tp4[:, j * P:(j + 1) * P],
                            lhsT=xts[j][:, kd * P:(kd + 1) * P],
                            rhs=ident, is_transpose=True,
                            start=(j == 0), stop=(j == 3),
                        )
                    copy_cast(xt[:, kd, g * 512:(g + 1) * 512], tp4.bitcast(FP32))

            wT = {}  # ('q'|'k', tt, h) -> [P, 512] bf16 tile

            def w_transpose_half(which, h):
                pairs = [w_pend.pop((which, 2 * h)), w_pend.pop((which, 2 * h + 1))]
                for tt in range(8):
                    tp4 = ps.tile([P, 512], FP32R, tag="ps")
                    for j in range(4):
                        nc.tensor.matmul(
                            tp4[:, j * P:(j + 1) * P],
                            lhsT=pairs[j // 2][:, j % 2, tt * P:(tt + 1) * P],
                            rhs=ident, is_transpose=True,
                            start=(j == 0), stop=(j == 3),
                        )
                    dst = sh5.tile([P, 512], BF16, tag="sh5")
                    dv = dst.rearrange("ki (pr kk two) -> ki pr two kk",
                                       pr=2, two=2)
                    for j in range(4):
                        copy_cast(dv[:, j // 2, j % 2, :],
                                  tp4[:, j * P:(j + 1) * P].bitcast(FP32))
                    wT[(which, tt, h)] = dst

            mT = {}  # (dt, half) -> [P, 512] bf16: M[dt-rows, 512h:512h+512]

            def m_compute(ec, dts=range(8)):
                for dt_ in dts:
                    h, q = dt_ // 4, dt_ % 4
                    mps = ps.tile([P, 512], FP32, tag="ps")
                    for tt in range(8):
                        nc.tensor.matmul(
                            mps,
                            lhsT=wT[("q", tt, h)][:, q * P:(q + 1) * P],
                            rhs=wT[("k", tt, ec)],
                            start=(tt == 0), stop=(tt == 7),
                        )
                    dst = sh5.tile([P, 512], BF16, tag="sh5")
                    copy_cast(dst, mps)
                    mT[(dt_, ec)] = dst

            def at_chunk(sc):
                for et in range(8):
                    aps = ps.tile([P, 512], FP32, tag="ps")
                    for dt_ in range(8):
                        nc.tensor.matmul(
                            aps,
                            lhsT=mT[(dt_, et // 4)][:, (et % 4) * P:(et % 4 + 1) * P],
                            rhs=xt[:, dt_, sc * 512:(sc + 1) * 512],
                            start=(dt_ == 0), stop=(dt_ == 7),
                        )
                    copy_cast(at[:, et, sc * 512:(sc + 1) * 512], aps)

            def wv_cast(kd2):
                t = w_pend.pop(("v", kd2))
                nc.gpsimd.tensor_copy(out=wv16[:, kd2 * 2:kd2 * 2 + 2, :], in_=t)

            def v_group(kts):
                for kt in kts:
                    for ec in range(2):
                        vps = ps.tile([P, 512], FP32, tag="ps")
                        for kd in range(8):
                            nc.tensor.matmul(
                                vps,
                                lhsT=xt[:, kd, kt * P:(kt + 1) * P],
                                rhs=wv16[:, kd, ec * 512:(ec + 1) * 512],
                                start=(kd == 0), stop=(kd == 7),
                            )
                        copy_cast(vsb[:, kt, ec * 512:(ec + 1) * 512], vps)

            est = {}
            esum = {}

            def s_blocks(c, ks):
                for k in ks:
                    j = k - 4 * c
                    w0 = max(0, j) * P  # diagonal blocks: skip always-invalid cols
                    sps = ps.tile([P, 512], FP32, tag="ps")
                    for e in range(8):
                        nc.tensor.matmul(
                            sps[:, w0:512],
                            lhsT=xt[:, e, k * P:(k + 1) * P],
                            rhs=at[:, e, c * 512 + w0:(c + 1) * 512],
                            start=(e == 0), stop=(e == 7),
                        )
                    et_ = sh5.tile([P, 512], BF16, tag="sh5")
                    nc.scalar.activation(
                        out=et_[:, w0:512], in_=sps[:, w0:512],
                        func=AF.Exp, scale=0.03125,
                    )
                    if w0 > 0:
                        nc.vector.memset(et_[:, 0:w0], 0.0)
                    if j >= 0:
                        nc.vector.tensor_mul(
                            et_[:, j * P:(j + 1) * P],
                            et_[:, j * P:(j + 1) * P], tri16,
                        )
                    est[(c, k)] = et_
                    # chunk-wide running sum of P^T rows (exact: invalid
                    # entries are zeroed) -> one rowsum matmul per q-tile
                    if c not in esum:
                        es = sh5.tile([P, 512], BF16, tag="sh5")
                        nc.vector.tensor_copy(out=es, in_=et_)
                        esum[c] = es
                    else:
                        nc.vector.tensor_add(esum[c], esum[c], et_)

            out_i = [0]

            def av_stage(c, js=range(4)):
                for j in js:
                    q_abs = 4 * c + j
                    o_ps0 = ps.tile([P, 512], FP32, tag="ps")
                    o_ps1 = ps.tile([P, 512], FP32, tag="ps")
                    rs_ps = rsp.tile([P, 2], FP32, tag="rs")
                    nc.tensor.matmul(rs_ps, lhsT=esum[c][:, j * P:(j + 1) * P],
                                     rhs=ones16, start=True, stop=True)
                    rec = rcp.tile([P, 1], FP32, tag="rcp")
                    nc.vector.reciprocal(rec, rs_ps[:, 0:1])
                    for k in range(q_abs + 1):
                        lw = est[(c, k)][:, j * P:(j + 1) * P]
                        st, sp = (k == 0), (k == q_abs)
                        nc.tensor.matmul(o_ps0, lhsT=lw, rhs=vsb[:, k, 0:512],
                                         start=st, stop=sp)
                        nc.tensor.matmul(o_ps1, lhsT=lw, rhs=vsb[:, k, 512:1024],
                                         start=st, stop=sp)
                    o_sb = s4.tile([P, D], FP32, tag="s4")
                    for h in range(2):
                        nc.vector.tensor_scalar_mul(
                            o_sb[:, h * 512:(h + 1) * 512],
                            (o_ps0, o_ps1)[h], rec,
                        )
                        out_i[0] += 1
                        (nc.gpsimd, nc.sync)[out_i[0] % 2].dma_start(
                            out=out_d[q_abs * P:(q_abs + 1) * P,
                                      h * 512:(h + 1) * 512],
                            in_=o_sb[:, h * 512:(h + 1) * 512],
                        )

            # ---- DMA issue order ----
            # Round-robin across the three rings so global arrival order
            # tracks PE consumption: Wq first (its transposes are the first
            # post-warmup PE work), then x interleaved with Wk, Wv last
            # (V projection runs late, interleaved into the S/AV phase).
            qs = (nc.sync, nc.scalar, nc.gpsimd)
            ticket = [0]

            def rr():
                t = ticket[0]
                ticket[0] += 1
                return qs[t % 3]

            load_w("q", 0, rr())
            load_w("q", 1, rr())
            load_w("k", 0, rr())
            load_w("k", 1, rr())
            load_w("q", 2, rr())
            load_w("q", 3, rr())
            load_w("k", 2, rr())
            load_w("k", 3, rr())
            for si in range(0, 16):
                load_x(si, rr())
            for kd2 in range(4):
                load_w("v", kd2, rr())

            # ---- PE schedule ----
            warmup(6)
            w_transpose_half("q", 0)
            w_transpose_half("k", 0)
            m_compute(0, range(0, 4))
            w_transpose_half("q", 1)
            m_compute(0, range(4, 8))
            w_transpose_half("k", 1)
            m_compute(1)
            x_transpose_group(0)
            at_chunk(0)
            x_transpose_group(1)
            at_chunk(1)
            x_transpose_group(2)
            at_chunk(2)
            x_transpose_group(3)
            at_chunk(3)
            for kd2 in range(4):
                wv_cast(kd2)
            v_group(range(0, 4))
            s_blocks(0, range(0, 4))
            v_group(range(4, 8))
            s_blocks(1, range(0, 8))
            av_stage(0)
            v_group(range(8, 12))
            s_blocks(2, range(0, 12))
            av_stage(1)
            v_group(range(12, 16))
            s_blocks(3, range(0, 8))
            av_stage(2)
            s_blocks(3, range(8, 16))
            av_stage(3)

    nc.compile()
    return nc


def _get_nc():
    global _built
    if _built is None:
        _built = _build()
    return _built


def _run(inputs, trace=False):
    from concourse.bass_utils import run_bass_kernel_spmd

    x = inputs["x"]
    in_maps = [
        {
            "x": np.ascontiguousarray(x[c], dtype=np.float32),
            "Wq": np.asarray(inputs["Wq"], dtype=np.float32),
            "Wk": np.asarray(inputs["Wk"], dtype=np.float32),
            "Wv": np.asarray(inputs["Wv"], dtype=np.float32),
        }
        for c in range(NCORES)
    ]
    res = run_bass_kernel_spmd(
        nc=_get_nc(), in_maps=in_maps, core_ids=list(range(NCORES)), trace=trace
    )
    out = np.stack([res.results[c]["out"] for c in range(NCORES)], axis=0)
    return out, res


def kernel(x, Wq, Wk, Wv):
    out, _ = _run({"x": x, "Wq": Wq, "Wk": Wk, "Wv": Wv}, trace=False)
    return out


# revision 26
# speedup vs baseline: 1.0006x; 1.0006x over previous
"""Causal single-head attention (B=8, S=2048, D=1024, fp32) on 8 NeuronCores.

Data-parallel over batch: one batch element per core, weights replicated.

Key algebraic trick: S = Q K^T = x (Wq Wk^T) x^T = A x^T with A = x M and
M = Wq Wk^T (1024x1024, computed once per core, ~2.1 GFLOP vs 4.3 GFLOP for
the K projection it replaces). K/Q are never materialized, and everything
(x^T, A^T, V) stays SBUF-resident in bf16 — no DRAM scratch traffic at all.

Per-core pipeline (matmuls bf16 @ 1 cycle/row, transposes fp32r):
  1. xT = x^T, WqT = Wq^T, WkT = Wk^T via PE transposes (cast bf16 on copy-out)
  2. M[d,e] = sum_t Wq[d,t] Wk[e,t]    (lhsT=WqT slices, rhs=WkT)
  3. AT = (x M)^T  (lhsT=M slices, rhs=xT) ; V = x Wv (lhsT=xT, rhs=Wv bf16)
  4. per 512-wide query chunk c, k-tile block:
       S^T[k, q-chunk] over 8 e-tiles of xT/AT (diagonal blocks width-trimmed)
       P^T = exp(S^T / 32) on ScalarE -> bf16; diagonal 128x128 triangle
       masked by a 0/1 multiply on DVE
       per q-tile j: rowsum / O accumulated interleaved (same est lhsT for
       O-halves and rowsum), O = (P^T.T @ V) * (1/rowsum)
"""

import numpy as np

B, S, D = 8, 2048, 1024
P = 128
NCORES = 8

_built = None


def _build():
    import ml_dtypes
    import concourse.tile as tile
    import concourse.mybir as mybir
    from concourse import bacc

    FP32 = mybir.dt.float32
    FP32R = mybir.dt.float32r
    BF16 = mybir.dt.bfloat16
    AF = mybir.ActivationFunctionType

    nc = bacc.Bacc("TRN2", target_bir_lowering=False, debug=False, num_devices=NCORES)
    x_d = nc.dram_tensor("x", [S, D], FP32R, kind="ExternalInput").ap()
    wq_d = nc.dram_tensor("Wq", [D, D], FP32R, kind="ExternalInput").ap()
    wk_d = nc.dram_tensor("Wk", [D, D], FP32R, kind="ExternalInput").ap()
    wv_d = nc.dram_tensor("Wv", [D, D], FP32, kind="ExternalInput").ap()
    out_d = nc.dram_tensor("out", [S, D], FP32, kind="ExternalOutput").ap()

    ident_c = nc.inline_tensor(np.eye(P, dtype=np.float32), name="ident_c")
    # [tri01 (128) | ones (2) | warm data (512)] in bf16
    tri = (np.arange(P)[None, :] >= np.arange(P)[:, None]).astype(np.float32)
    cst_np = np.concatenate([tri, np.ones((P, 2 + 512), np.float32)], axis=1)
    cst_c = nc.inline_tensor(cst_np.astype(ml_dtypes.bfloat16), name="cst_c")

    with tile.TileContext(nc) as tc:
        with (
            tc.tile_pool(name="bigs", bufs=1) as bigs,
            tc.tile_pool(name="sh5", bufs=40) as sh5,    # wqT/wkT, M, then est ring
            tc.tile_pool(name="s4", bufs=5) as s4,       # x staging, then osb
            tc.tile_pool(name="wst", bufs=4) as wst,     # W fp32 staging (pairs)
            tc.tile_pool(name="smalls", bufs=1) as smalls,
            tc.tile_pool(name="rcp", bufs=2) as rcp,
            tc.tile_pool(name="ps", bufs=6, space="PSUM") as ps,
            tc.tile_pool(name="rsp", bufs=2, space="PSUM") as rsp,
        ):
            ident = smalls.tile([P, P], FP32R, tag="ident")
            nc.sync.dma_start(out=ident, in_=ident_c.ap().bitcast(FP32R))
            cst = smalls.tile([P, 642], BF16, tag="cst")
            nc.scalar.dma_start(out=cst, in_=cst_c.ap())
            tri16 = cst[:, 0:128]
            ones16 = cst[:, 128:130]
            warm16 = cst[:, 130:642]

            xt = bigs.tile([P, 8, S], BF16, tag="xt")
            at = bigs.tile([P, 8, S], BF16, tag="at")
            vsb = bigs.tile([P, 16, D], BF16, tag="vsb")
            wv16 = bigs.tile([P, 8, D], BF16, tag="wv16")

            # ---- input DMA issue (3 queues) ----
            x_pend = {}
            w_pend = {}

            def load_x(si, eng):
                t = s4.tile([P, D], FP32R, tag="s4")
                eng.dma_start(out=t, in_=x_d[si * P:(si + 1) * P, :])
                x_pend[si] = t

            def load_w(which, kd2, eng):
                src, dt_ = {
                    "q": (wq_d, FP32R), "k": (wk_d, FP32R), "v": (wv_d, FP32),
                }[which]
                t = wst.tile([P, 2, D], dt_, tag="wst")
                # Wq/Wk: each partition gets two contiguous DRAM rows (8KB
                # descriptors, 2x queue rate); the d-interleave is undone in
                # the transpose copy-outs. Wv keeps the plain layout.
                pat = "(ki two) t -> ki two t" if which != "v" else \
                    "(two ki) t -> ki two t"
                eng.dma_start(
                    out=t,
                    in_=src[kd2 * 2 * P:(kd2 + 1) * 2 * P, :].rearrange(
                        pat, two=2
                    ) if which != "v" else
                    src[kd2 * 2 * P:(kd2 + 1) * 2 * P, :].rearrange(
                        pat, ki=P
                    ),
                )
                w_pend[(which, kd2)] = t

            # copy engines alternate
            cp_i = [0]

            def cp_eng():
                cp_i[0] += 1
                return (nc.vector, nc.scalar)[cp_i[0] % 2]

            def copy_cast(out, in_):
                e = cp_eng()
                if e is nc.vector:
                    e.tensor_copy(out=out, in_=in_)
                else:
                    e.copy(out=out, in_=in_)

            # ---- PE phases ----
            def warmup(rounds):
                for _ in range(rounds):
                    dps = ps.tile([P, 512], FP32, tag="ps")
                    for r in range(8):
                        nc.tensor.matmul(
                            dps, lhsT=tri16, rhs=warm16,
                            start=(r == 0), stop=(r == 7),
                        )

            def x_transpose_group(g):
                xts = [x_pend.pop(si) for si in range(4 * g, 4 * g + 4)]
                for kd in range(8):
                    tp4 = ps.tile([P, 512], FP32R, tag="ps")
                    for j in range(4):
                        nc.tensor.matmul(
                            tp4[:, j * P:(j + 1) * P],
                            lhsT=xts[j][:, kd * P:(kd + 1) * P],
                            rhs=ident, is_transpose=True,
                            start=(j == 0), stop=(j == 3),
                        )
                    copy_cast(xt[:, kd, g * 512:(g + 1) * 512], tp4.bitcast(FP32))

            wT = {}  # ('q'|'k', tt, h) -> [P, 512] bf16 tile

            def w_transpose_half(which, h):
                pairs = [w_pend.pop((which, 2 * h)), w_pend.pop((which, 2 * h + 1))]
                for tt in range(8):
                    tp4 = ps.tile([P, 512], FP32R, tag="ps")
                    for j in range(4):
                        nc.tensor.matmul(
                            tp4[:, j * P:(j + 1) * P],
                            lhsT=pairs[j // 2][:, j % 2, tt * P:(tt + 1) * P],
                            rhs=ident, is_transpose=True,
                            start=(j == 0), stop=(j == 3),
                        )
                    dst = sh5.tile([P, 512], BF16, tag="sh5")
                    dv = dst.rearrange("ki (pr kk two) -> ki pr two kk",
                                       pr=2, two=2)
                    for j in range(4):
                        copy_cast(dv[:, j // 2, j % 2, :],
                                  tp4[:, j * P:(j + 1) * P].bitcast(FP32))
                    wT[(which, tt, h)] = dst

            mT = {}  # (dt, half) -> [P, 512] bf16: M[dt-rows, 512h:512h+512]

            def m_compute(ec, dts=range(8)):
                for dt_ in dts:
                    h, q = dt_ // 4, dt_ % 4
                    mps = ps.tile([P, 512], FP32, tag="ps")
                    for tt in range(8):
                        nc.tensor.matmul(
                            mps,
                            lhsT=wT[("q", tt, h)][:, q * P:(q + 1) * P],
                            rhs=wT[("k", tt, ec)],
                            start=(tt == 0), stop=(tt == 7),
                        )
                    dst = sh5.tile([P, 512], BF16, tag="sh5")
                    copy_cast(dst, mps)
                    mT[(dt_, ec)] = dst

            def at_chunk(sc):
                for et in range(8):
                    aps = ps.tile([P, 512], FP32, tag="ps")
                    for dt_ in range(8):
                        nc.tensor.matmul(
                            aps,
                            lhsT=mT[(dt_, et // 4)][:, (et % 4) * P:(et % 4 + 1) * P],
                            rhs=xt[:, dt_, sc * 512:(sc + 1) * 512],
                            start=(dt_ == 0), stop=(dt_ == 7),
                        )
                    copy_cast(at[:, et, sc * 512:(sc + 1) * 512], aps)

            def wv_cast(kd2):
                t = w_pend.pop(("v", kd2))
                nc.gpsimd.tensor_copy(out=wv16[:, kd2 * 2:kd2 * 2 + 2, :], in_=t)

            def v_group(kts):
                for kt in kts:
                    for ec in range(2):
                        vps = ps.tile([P, 512], FP32, tag="ps")
                        for kd in range(8):
                            nc.tensor.matmul(
                                vps,
                                lhsT=xt[:, kd, kt * P:(kt + 1) * P],
                                rhs=wv16[:, kd, ec * 512:(ec + 1) * 512],
                                start=(kd == 0), stop=(kd == 7),
                            )
                        copy_cast(vsb[:, kt, ec * 512:(ec + 1) * 512], vps)

            est = {}
            esum = {}

            def s_blocks(c, ks):
                for k in ks:
                    j = k - 4 * c
                    w0 = max(0, j) * P  # diagonal blocks: skip always-invalid cols
                    sps = ps.tile([P, 512], FP32, tag="ps")
                    for e in range(8):
                        nc.tensor.matmul(
                            sps[:, w0:512],
                            lhsT=xt[:, e, k * P:(k + 1) * P],
                            rhs=at[:, e, c * 512 + w0:(c + 1) * 512],
                            start=(e == 0), stop=(e == 7),
                        )
                    et_ = sh5.tile([P, 512], BF16, tag="sh5")
                    nc.scalar.activation(
                        out=et_[:, w0:512], in_=sps[:, w0:512],
                        func=AF.Exp, scale=0.03125,
                    )
                    if w0 > 0:
                        nc.vector.memset(et_[:, 0:w0], 0.0)
                    if j >= 0:
                        nc.vector.tensor_mul(
                            et_[:, j * P:(j + 1) * P],
                            et_[:, j * P:(j + 1) * P], tri16,
                        )
                    est[(c, k)] = et_
                    # chunk-wide running sum of P^T rows (exact: invalid
                    # entries are zeroed) -> one rowsum matmul per q-tile
                    if c not in esum:
                        es = sh5.tile([P, 512], BF16, tag="sh5")
                        nc.vector.tensor_copy(out=es, in_=et_)
                        esum[c] = es
                    else:
                        nc.vector.tensor_add(esum[c], esum[c], et_)

            out_i = [0]

            def av_stage(c, js=range(4)):
                for j in js:
                    q_abs = 4 * c + j
                    o_ps0 = ps.tile([P, 512], FP32, tag="ps")
                    o_ps1 = ps.tile([P, 512], FP32, tag="ps")
                    rs_ps = rsp.tile([P, 2], FP32, tag="rs")
                    nc.tensor.matmul(rs_ps, lhsT=esum[c][:, j * P:(j + 1) * P],
                                     rhs=ones16, start=True, stop=True)
                    rec = rcp.tile([P, 1], FP32, tag="rcp")
                    nc.vector.reciprocal(rec, rs_ps[:, 0:1])
                    for k in range(q_abs + 1):
                        lw = est[(c, k)][:, j * P:(j + 1) * P]
                        st, sp = (k == 0), (k == q_abs)
                        nc.tensor.matmul(o_ps0, lhsT=lw, rhs=vsb[:, k, 0:512],
                                         start=st, stop=sp)
                        nc.tensor.matmul(o_ps1, lhsT=lw, rhs=vsb[:, k, 512:1024],
                                         start=st, stop=sp)
                    o_sb = s4.tile([P, D], FP32, tag="s4")
                    for h in range(2):
                        nc.vector.tensor_scalar_mul(
                            o_sb[:, h * 512:(h + 1) * 512],
                            (o_ps0, o_ps1)[h], rec,
                        )
                        out_i[0] += 1
                        (nc.gpsimd, nc.sync)[out_i[0] % 2].dma_start(
                            out=out_d[q_abs * P:(q_abs + 1) * P,
                                      h * 512:(h + 1) * 512],
                            in_=o_sb[:, h * 512:(h + 1) * 512],
                        )

            # ---- DMA issue order ----
            # Round-robin across the three rings so global arrival order
            # tracks PE consumption: Wq first (its transposes are the first
            # post-warmup PE work), then x interleaved with Wk, Wv last
            # (V projection runs late, interleaved into the S/AV phase).
            qs = (nc.sync, nc.scalar, nc.gpsimd)
            ticket = [0]

            def rr():
                t = ticket[0]
                ticket[0] += 1
                return qs[t % 3]

            load_w("q", 0, rr())
            load_w("q", 1, rr())
            load_w("k", 0, rr())
            load_w("k", 1, rr())
            load_w("q", 2, rr())
            load_w("q", 3, rr())
            load_w("k", 2, rr())
            load_w("k", 3, rr())
            for kd2 in range(4):
                load_w("v", kd2, rr())
            for si in range(0, 16):
                load_x(si, rr())

            # ---- PE schedule ----
            warmup(6)
            w_transpose_half("q", 0)
            w_transpose_half("k", 0)
            m_compute(0, range(0, 4))
            w_transpose_half("q", 1)
            m_compute(0, range(4, 8))
            w_transpose_half("k", 1)
            m_compute(1)
            for kd2 in range(4):
                wv_cast(kd2)
            x_transpose_group(0)
            at_chunk(0)
            v_group(range(0, 4))
            x_transpose_group(1)
            at_chunk(1)
            v_group(range(4, 8))
            x_transpose_group(2)
            at_chunk(2)
            v_group(range(8, 12))
            x_transpose_group(3)
            at_chunk(3)
            s_blocks(0, range(0, 4))
            s_blocks(1, range(0, 8))
            av_stage(0)
            v_group(range(12, 16))
            s_blocks(2, range(0, 12))
            av_stage(1)
            s_blocks(3, range(0, 8))
            av_stage(2)
            s_blocks(3, range(8, 16))
            av_stage(3)

    nc.compile()
    return nc


def _get_nc():
    global _built
    if _built is None:
        _built = _build()
    return _built


def _run(inputs, trace=False):
    from concourse.bass_utils import run_bass_kernel_spmd

    x = inputs["x"]
    in_maps = [
        {
            "x": np.ascontiguousarray(x[c], dtype=np.float32),
            "Wq": np.asarray(inputs["Wq"], dtype=np.float32),
            "Wk": np.asarray(inputs["Wk"], dtype=np.float32),
            "Wv": np.asarray(inputs["Wv"], dtype=np.float32),
        }
        for c in range(NCORES)
    ]
    res = run_bass_kernel_spmd(
        nc=_get_nc(), in_maps=in_maps, core_ids=list(range(NCORES)), trace=trace
    )
    out = np.stack([res.results[c]["out"] for c in range(NCORES)], axis=0)
    return out, res


def kernel(x, Wq, Wk, Wv):
    out, _ = _run({"x": x, "Wq": Wq, "Wk": Wk, "Wv": Wv}, trace=False)
    return out
